# revision 26
# baseline (speedup 1.0000x reference)
"""ColorAttention Trainium2 kernel (v2: fp8 mask stream + DoubleRow reduce).

Data-parallel over batch: core b handles batch element b.
Per core:
  - mask [256,256,313] is sent as fp8e4 (0/1 exact, 20MB instead of 82MB)
    and patch-reduced on the PE with DoubleRow fp8 matmuls: image rows
    (r, r+64) form the two interleaved k-tiles, block-diagonal bd2
    stationaries map 128 rows -> 128 patch sums at 0.5 cyc/col.
    Multiplicative attention mask is_one(m) = relu(1-(m-1)^2).
  - attention computed in transposed layout throughout:
      qkvT[f,n] = sum_e qkv_wT[e,f] * inputsT[e,n]
      scoresT[m,n] = sum_d kT[d,m] qT[d,n];  expT = exp(scoresT/tau) * mask
      outT_aug[d|1,n] = sum_m v_aug[m,d|1] expT[m,n]   (row 64 = denom)
      out[n,g] = (sum_{h,d} (outT_h/denom_h)[d,n] o_wT[h*64+d,g]) + o_b
  - o_b is folded into the o_proj psum via a K=1 ones-row matmul.
"""

import os
import numpy as np
import ml_dtypes

# tolerate environments without the optional NTFF profile hook module when
# BASS_TRACE is set externally
try:
    import antenv.axon_hooks  # noqa: F401
except Exception:
    import sys as _sys
    import types as _types
    _m = _types.ModuleType("antenv.axon_hooks")
    _m.set_axon_ntff_profile_hook = lambda h: None
    _m.get_axon_ntff_profile_hook = lambda: None
    try:
        import antenv
        antenv.axon_hooks = _m
        _sys.modules["antenv.axon_hooks"] = _m
    except Exception:
        pass

import concourse.bass as bass
import concourse.mybir as mybir
import concourse.tile as tile
from concourse import bacc
from concourse.bass_utils import run_bass_kernel_spmd

F32 = mybir.dt.float32
F32R = mybir.dt.float32r
BF16 = mybir.dt.bfloat16
F8E4 = mybir.dt.float8e4
AFT = mybir.ActivationFunctionType
DR = mybir.MatmulPerfMode.DoubleRow

B = 8
SEQ = 256
NCLS = 313
E = 512
NH = 8
HD = 64
N1 = SEQ + NCLS  # 569
P = 16
IMG = 256

# n/m chunking of the 569 token dim.
# fp32r matmuls require even free counts, so padded widths (..P) are used for
# fp32r operands/psum; real widths for bf16 ops and final stores.
N1P = 570
CHUNKS = [(0, 128), (128, 128), (256, 128), (384, 128), (512, 57)]
CWP = [128, 128, 128, 128, 58]
SPANS = [(0, 512), (512, 58)]

LAST_RESULT = None
_CACHED = {}


def r32(ap):
    if ap.dtype == F32R:
        return ap
    return ap.bitcast(F32R)


def _build_program():
    nc = bacc.Bacc("TRN2", target_bir_lowering=False, debug=False, num_devices=B)

    # ---- DRAM I/O ----
    d_xT = nc.dram_tensor("xT", [E, N1], F32R, kind="ExternalInput").ap()
    d_mask = nc.dram_tensor("mask8", [IMG, IMG * NCLS], F8E4, kind="ExternalInput").ap()
    d_qkvwT = nc.dram_tensor("qkv_wT", [E, 3 * E], F32R, kind="ExternalInput").ap()
    d_owT = nc.dram_tensor("o_wT", [E, E], F32R, kind="ExternalInput").ap()
    d_ob = nc.dram_tensor("o_b", [1, E], F32R, kind="ExternalInput").ap()
    d_tau = nc.dram_tensor("tau", [1, 1], F32, kind="ExternalInput").ap()
    d_bd2 = nc.dram_tensor("bd2", [128, 16 * 2 * 128], F8E4, kind="ExternalInput").ap()
    d_ident = nc.dram_tensor("ident", [128, 128], BF16, kind="ExternalInput").ap()
    d_unitv = nc.dram_tensor("unitv", [128, 512], F32R, kind="ExternalInput").ap()
    d_out = nc.dram_tensor("out", [N1, E], F32, kind="ExternalOutput").ap()

    with tile.TileContext(nc) as tc:
        _emit(nc, tc, d_xT, d_mask, d_qkvwT, d_owT, d_ob, d_tau, d_bd2, d_ident,
              d_unitv, d_out)

    nc.compile()
    return nc


def _emit(nc, tc, d_xT, d_mask, d_qkvwT, d_owT, d_ob, d_tau, d_bd2, d_ident,
          d_unitv, d_out):
    from contextlib import ExitStack

    ctx = ExitStack()
    singles = ctx.enter_context(tc.tile_pool(name="singles", bufs=1))
    expool = ctx.enter_context(tc.tile_pool(name="expT", bufs=40))
    opool = ctx.enter_context(tc.tile_pool(name="outTsb", bufs=8))
    spool = ctx.enter_context(tc.tile_pool(name="smalls", bufs=2))
    ps_work = ctx.enter_context(tc.tile_pool(name="ps_work", bufs=4, space="PSUM"))
    mctx = ExitStack()
    mpool = mctx.enter_context(tc.tile_pool(name="mask_stream", bufs=3))
    ps_mask = mctx.enter_context(tc.tile_pool(name="ps_mask", bufs=2, space="PSUM"))

    # ---- persistent SBUF ----
    inputsT = [singles.tile([128, N1P], F32R, tag=f"inT{i}", name=f"inT{i}") for i in range(4)]
    qkvwT = [singles.tile([128, 3 * E], F32R, tag=f"qkvwT{i}", name=f"qkvwT{i}") for i in range(4)]
    owT = [singles.tile([64, E], F32R, tag=f"owT{i}", name=f"owT{i}") for i in range(8)]
    bd2_sb = singles.tile([128, 16, 2, 128], F8E4, tag="bd2", name="bd2_sb")
    ident_sb = singles.tile([128, 128], BF16, tag="ident", name="ident_sb")
    ones_sb = singles.tile([128, 128], F32R, tag="ones", name="ones_sb")
    unitv_sb = singles.tile([128, 512], F32R, tag="unitv", name="unitv_sb")
    rtau = singles.tile([128, 1], F32, tag="rtau", name="rtau")
    ob_sb = singles.tile([1, E], F32R, tag="ob", name="ob_sb")
    qkT = [singles.tile([128, N1P], F32R, tag=f"qkT{i}", name=f"qkT{i}") for i in range(8)]
    v_sb = [singles.tile([128, NH * (HD + 1)], BF16, tag=f"vsb{i}", name=f"v_sb{i}") for i in range(5)]
    isone = [singles.tile([128, NCLS], BF16, tag=f"iso{i}", name=f"isone{i}") for i in range(2)]
    isoT = [singles.tile([128, SEQ], BF16, tag=f"isoT{i}", name=f"isoT{i}") for i in range(3)]

    # ---- setup DMAs ----
    # bd2 first on the sync queue (first mask matmul gates on it), then the
    # attention operands.
    nc.sync.dma_start(out=bd2_sb, in_=d_bd2)
    nc.sync.dma_start(out=ident_sb, in_=d_ident)
    for i in range(4):
        nc.sync.dma_start(out=inputsT[i][:, :N1], in_=d_xT[i * 128:(i + 1) * 128, :])
        nc.vector.memset(inputsT[i][:, N1:N1P].bitcast(F32), 0.0)
        nc.sync.dma_start(out=qkvwT[i], in_=d_qkvwT[i * 128:(i + 1) * 128, :])
    for h in range(8):
        nc.sync.dma_start(out=owT[h], in_=d_owT[h * 64:(h + 1) * 64, :])
    nc.sync.dma_start(out=unitv_sb, in_=d_unitv)
    nc.sync.dma_start(out=ob_sb, in_=d_ob)
    nc.vector.memset(ones_sb[:].bitcast(F32), 1.0)
    # broadcast tau to all partitions (step-0 partition AP), then reciprocal
    tau_bc = bass.AP(tensor=d_tau.tensor, offset=d_tau.offset, ap=[[0, 128], [1, 1]])
    tau_sb = singles.tile([128, 1], F32, tag="tau", name="tau_sb")
    nc.scalar.dma_start(out=tau_sb, in_=tau_bc)
    nc.vector.reciprocal(out=rtau, in_=tau_sb)
    neg1 = singles.tile([128, 1], F32, tag="neg1", name="neg1")
    nc.vector.memset(neg1, -1.0)

    # ---- HAM warmup: dense dummy matmuls so the PE p-state ramps toward
    # full clock while the first mask tiles land; garbage to a scratch psum
    # that is never read ----
    scr = singles.tile([128, 640], BF16, tag="scr", name="scr")
    nc.vector.memset(scr, 1.0)
    ps_warm = ps_work.tile([128, 512], F32, tag="pswork", name="ps_warm")
    for _ in range(20):
        nc.tensor.matmul(out=ps_warm, lhsT=scr[:, 0:128], rhs=scr[:, 128:640],
                         start=True, stop=True)

    # ---- attention work units (emitted interleaved with the mask stream) ----
    expT = {}

    def unit_qkvT(fc):
        def go():
            for sp, (s0, sw) in enumerate(SPANS):
                ps = ps_work.tile([128, sw], F32, tag="pswork", name="pswork")
                for ec in range(4):
                    nc.tensor.matmul(
                        out=ps,
                        lhsT=r32(qkvwT[ec][:, fc * 128:(fc + 1) * 128]),
                        rhs=r32(inputsT[ec][:, s0:s0 + sw]),
                        start=(ec == 0), stop=(ec == 3),
                    )
                nc.vector.tensor_copy(out=qkT[fc][:, s0:s0 + sw], in_=ps)
        return go

    def unit_v(mc):
        def go():
            c0, cw = CHUNKS[mc]
            cwp = CWP[mc]
            ps = ps_work.tile([128, E], F32, tag="pswork", name="pswork")
            for ec in range(4):
                nc.tensor.matmul(
                    out=ps[:cwp, :],
                    lhsT=r32(inputsT[ec][:, c0:c0 + cwp]),
                    rhs=r32(qkvwT[ec][:, 2 * E:3 * E]),
                    start=(ec == 0), stop=(ec == 3),
                )
            for h in range(NH):
                nc.vector.tensor_copy(
                    out=v_sb[mc][:cw, h * 65:h * 65 + 64],
                    in_=ps[:cw, h * 64:(h + 1) * 64],
                )
            nc.vector.memset(v_sb[mc][:cw, 64::65], 1.0)
        return go

    def unit_scores(h, mc):
        def go():
            c0, cw = CHUNKS[mc]
            cwp = CWP[mc]
            kt = qkT[4 + h // 2]
            qt = qkT[h // 2]
            hb = 64 * (h % 2)
            et = expool.tile([128, N1P], BF16, tag="expT", name="expT")
            expT[(h, mc)] = et
            for sp, (s0, sw) in enumerate(SPANS):
                ps = ps_work.tile([128, sw], F32, tag="pswork", name="pswork")
                nc.tensor.matmul(
                    out=ps[:cwp, :],
                    lhsT=r32(kt[hb:hb + 64, c0:c0 + cwp]),
                    rhs=r32(qt[hb:hb + 64, s0:s0 + sw]),
                    start=True, stop=True,
                )
                nc.scalar.activation(
                    out=et[:cwp, s0:s0 + sw], in_=ps[:cwp, :],
                    func=AFT.Exp, scale=rtau[:cwp],
                )
        return go

    def unit_mult(h, mc):
        def go():
            c0, cw = CHUNKS[mc]
            et = expT[(h, mc)]
            if mc < 2:
                nc.vector.tensor_mul(
                    out=et[:cw, SEQ:N1], in0=et[:cw, SEQ:N1], in1=isone[mc])
            else:
                nc.vector.tensor_mul(
                    out=et[:cw, 0:SEQ], in0=et[:cw, 0:SEQ], in1=isoT[mc - 2][:cw, :])
        return go

    units = [unit_qkvT(fc) for fc in range(8)]
    units += [unit_v(mc) for mc in range(5)]
    units += [unit_scores(h, mc) for h in range(NH) for mc in range(5)]

    # ---- is_one computation (psum -> multiplicative mask) ----
    ps_m = [None, None]

    def emit_isone(i):
        tmp = spool.tile([128, NCLS], F32, tag="isotmp", name="isotmp")
        nc.scalar.activation(out=tmp, in_=ps_m[i], func=AFT.Square, bias=neg1)
        nc.scalar.activation(out=isone[i], in_=tmp, func=AFT.Relu, scale=-1.0, bias=1.0)

    # transpose one is_one half -> isoT columns (c on partitions); emitted
    # mid-stream for i=0, post-stream for i=1
    def emit_isoT(i):
        for j in range(3):
            cw = 57 if j == 2 else 128
            pst = ps_work.tile([128, 128], BF16, tag="pswork", name="pswork_t")
            nc.tensor.transpose(out=pst[:cw, :], in_=isone[i][:, j * 128:j * 128 + cw],
                                identity=ident_sb)
            nc.vector.tensor_copy(out=isoT[j][:cw, i * 128:(i + 1) * 128], in_=pst[:cw, :])

    # ---- the mask stream ----
    # 32 fp8 tiles of [128 rows x 16q x 313c] (0.64MB, 5KB contiguous
    # per-partition lines). The two DoubleRow k-tiles are ADJACENT pixel
    # columns (same patch column, identical bd mapping), so each tile needs
    # only 8 matmuls of K=128x2 into ps_m[rt]; each tile's matmuls share one
    # stationary bd2 variant.
    QO = 16
    n_q = IMG // QO  # 16
    ui = 0
    for rt in range(2):
        ps_m[rt] = ps_mask.tile([128, NCLS], F32, tag="psmask", name="psmask")
        for Q in range(n_q):
            t = mpool.tile([128, QO, NCLS], F8E4, tag="mstream", name="mstream")
            src = bass.AP(
                tensor=d_mask.tensor,
                offset=d_mask.offset + rt * 128 * IMG * NCLS + Q * QO * NCLS,
                ap=[[IMG * NCLS, 128], [1, QO * NCLS]],
            )
            if Q % 2 == 0:
                nc.gpsimd.dma_start(out=t, in_=src)
            else:
                nc.sync.dma_start(out=t, in_=src)
            for j in range(QO // 2):
                nc.tensor.matmul(
                    out=ps_m[rt],
                    lhsT=bd2_sb[:, Q],
                    rhs=t[:, 2 * j:2 * j + 2, :],
                    start=(Q == 0 and j == 0),
                    stop=(Q == n_q - 1 and j == QO // 2 - 1),
                    perf_mode=DR,
                )
            ti = rt * n_q + Q
            # keep the PE dense: from tile 4 onward interleave attention
            # units (their DMAs on the sync queue land ~12us in)
            if ti >= 4 and ui < len(units):
                budget = 2 if ti >= 8 else 1
                for _ in range(budget):
                    if ui < len(units):
                        units[ui]()
                        ui += 1
            if ti == 20:
                emit_isoT(0)
        emit_isone(rt)
    while ui < len(units):
        units[ui]()
        ui += 1
    emit_isoT(1)
    mctx.close()
    ps_out = ctx.enter_context(tc.tile_pool(name="ps_out", bufs=1, space="PSUM"))

    # ---- mask-mult + attn@v with gathered denominators ----
    # Per group of 4 heads: mask-mult expT, attn@v into psum (ones column of
    # v gives the softmax denominator in row 64), evacuate the unnormalized
    # outT to SBUF, and gather the 4 heads' denominator rows at partitions
    # {0,32,64,96} of a shared psum tile via K=1 unit-vector matmuls. Then a
    # single reciprocal per span serves the whole group; PE broadcasts each
    # head's reciprocal row and DVE normalizes outT in place (reading the
    # broadcast psum directly).
    outT = [opool.tile([64, N1P], F32R, tag="outT", name="outT") for _ in range(NH)]
    for g in range(2):
        den_ps = {}
        for sp, (s0, sw) in enumerate(SPANS):
            den_ps[sp] = ps_out.tile([128, sw], F32, tag=f"denps{sp}", name="denps", bufs=1)
        for h4 in range(4):
            h = g * 4 + h4
            for mc in range(5):
                unit_mult(h, mc)()
            rec = spool.tile([65, N1P], F32R, tag="rec", name="rec")
            for sp, (s0, sw) in enumerate(SPANS):
                pso = ps_out.tile([65, sw], F32, tag="psout", name="psout", bufs=2)
                for mc in range(5):
                    c0, cw = CHUNKS[mc]
                    nc.tensor.matmul(
                        out=pso,
                        lhsT=v_sb[mc][:cw, h * 65:(h + 1) * 65],
                        rhs=expT[(h, mc)][:cw, s0:s0 + sw],
                        start=(mc == 0), stop=(mc == 4),
                    )
                with nc.allow_low_precision(reason="f32r copies"):
                    nc.scalar.activation(out=rec[64:65, s0:s0 + sw], in_=pso[64:65, :],
                                         func=AFT.Copy)
                    nc.vector.tensor_copy(out=outT[h][:, s0:s0 + sw], in_=pso[0:64, :])
                nc.tensor.matmul(
                    out=den_ps[sp],
                    lhsT=r32(unitv_sb[64:65, h4 * 128:(h4 + 1) * 128]),
                    rhs=r32(rec[64:65, s0:s0 + sw]),
                    start=(h4 == 0), stop=(h4 == 3),
                )
        drec = {}
        for sp, (s0, sw) in enumerate(SPANS):
            dr = spool.tile([128, sw], F32R, tag=f"drec{sp}", name=f"drec{sp}")
            with nc.allow_low_precision(reason="f32r reciprocal"):
                nc.vector.reciprocal(out=dr, in_=den_ps[sp])
            drec[sp] = dr
        for h4 in range(4):
            h = g * 4 + h4
            for sp, (s0, sw) in enumerate(SPANS):
                psb = ps_work.tile([64, sw], F32, tag="pswork", name="psb")
                nc.tensor.matmul(
                    out=psb,
                    lhsT=r32(ones_sb[32 * h4:32 * h4 + 1, 0:64]),
                    rhs=drec[sp][32 * h4:32 * h4 + 1, :],
                    start=True, stop=True,
                    tile_position=(32 * h4, 0),
                )
                with nc.allow_low_precision(reason="in-place normalize"):
                    nc.vector.tensor_mul(
                        out=outT[h][:, s0:s0 + sw], in0=outT[h][:, s0:s0 + sw],
                        in1=psb)

    # ---- o_proj + bias + store (bias via K=1 ones-row matmul) ----
    for mc in range(5):
        c0, cw = CHUNKS[mc]
        cwp = CWP[mc]
        psf = ps_work.tile([128, E], F32, tag="pswork", name="psf")
        nc.tensor.matmul(
            out=psf[:cwp, :],
            lhsT=r32(ones_sb[0:1, 0:cwp]),
            rhs=ob_sb,
            start=True, stop=False,
        )
        for h in range(NH):
            nc.tensor.matmul(
                out=psf[:cwp, :],
                lhsT=r32(outT[h][:, c0:c0 + cwp]),
                rhs=r32(owT[h]),
                start=False, stop=(h == NH - 1),
            )
        fin = spool.tile([128, E], F32, tag="fin", name="fin")
        nc.scalar.activation(out=fin[:cw, :], in_=psf[:cw, :], func=AFT.Copy)
        nc.sync.dma_start(out=d_out[c0:c0 + cw, :], in_=fin[:cw, :])

    ctx.close()


def _constants():
    # bd2[w][r, kt, s'] = 1 iff s' = (r//16)*16 + w; both k-tiles (adjacent
    # pixel columns of the same patch) use the identical row->patch mapping.
    bd2 = np.zeros((16, 128, 2, 128), dtype=np.float32)
    r = np.arange(128)
    for w in range(16):
        for kt in range(2):
            bd2[w, r, kt, (r // 16) * 16 + w] = 1.0
    # flatten to the [128, 16*2*128] DRAM layout: partition r, then (w, kt, s')
    bd2 = bd2.transpose(1, 0, 2, 3).reshape(128, 16 * 2 * 128)
    ident = np.eye(128, dtype=ml_dtypes.bfloat16)
    unitv = np.zeros((128, 512), dtype=np.float32)
    for h4 in range(4):
        unitv[:, h4 * 128 + 32 * h4] = 1.0
    return bd2.astype(ml_dtypes.float8_e4m3), ident, unitv


def kernel(x, colors, mask, qkv_w, o_w, o_b, tau):
    global LAST_RESULT
    if "nc" not in _CACHED:
        _CACHED["nc"] = _build_program()
    nc = _CACHED["nc"]

    bd2, ident, unitv = _constants()
    qkv_wT = np.ascontiguousarray(np.asarray(qkv_w, dtype=np.float32).T)
    o_wT = np.ascontiguousarray(np.asarray(o_w, dtype=np.float32).T)
    o_b2 = np.asarray(o_b, dtype=np.float32).reshape(1, E)
    tau2 = np.asarray(tau, dtype=np.float32).reshape(1, 1)

    in_maps = []
    for b in range(B):
        xT = np.ascontiguousarray(
            np.concatenate([np.asarray(x[b]), np.asarray(colors[b])], axis=0).T
        ).astype(np.float32)
        mb = np.asarray(mask[b], dtype=np.float32).reshape(IMG, IMG * NCLS)
        mb8 = mb.astype(ml_dtypes.float8_e4m3)
        in_maps.append({
            "xT": xT, "mask8": mb8, "qkv_wT": qkv_wT, "o_wT": o_wT,
            "o_b": o_b2, "tau": tau2, "bd2": bd2, "ident": ident, "unitv": unitv,
        })

    res = run_bass_kernel_spmd(nc, in_maps, list(range(B)))
    LAST_RESULT = res
    out = np.stack([res.results[i]["out"] for i in range(B)]).astype(np.float32)
    return out


# revision 28
# speedup vs baseline: 1.0543x; 1.0543x over previous
"""ColorAttention Trainium2 kernel (v2: fp8 mask stream + DoubleRow reduce).

Data-parallel over batch: core b handles batch element b.
Per core:
  - mask [256,256,313] is sent as fp8e4 (0/1 exact, 20MB instead of 82MB)
    and patch-reduced on the PE with DoubleRow fp8 matmuls: image rows
    (r, r+64) form the two interleaved k-tiles, block-diagonal bd2
    stationaries map 128 rows -> 128 patch sums at 0.5 cyc/col.
    Multiplicative attention mask is_one(m) = relu(1-(m-1)^2).
  - attention computed in transposed layout throughout:
      qkvT[f,n] = sum_e qkv_wT[e,f] * inputsT[e,n]
      scoresT[m,n] = sum_d kT[d,m] qT[d,n];  expT = exp(scoresT/tau) * mask
      outT_aug[d|1,n] = sum_m v_aug[m,d|1] expT[m,n]   (row 64 = denom)
      out[n,g] = (sum_{h,d} (outT_h/denom_h)[d,n] o_wT[h*64+d,g]) + o_b
  - o_b is folded into the o_proj psum via a K=1 ones-row matmul.
"""

import os
import numpy as np
import ml_dtypes

# tolerate environments without the optional NTFF profile hook module when
# BASS_TRACE is set externally
try:
    import antenv.axon_hooks  # noqa: F401
except Exception:
    import sys as _sys
    import types as _types
    _m = _types.ModuleType("antenv.axon_hooks")
    _m.set_axon_ntff_profile_hook = lambda h: None
    _m.get_axon_ntff_profile_hook = lambda: None
    try:
        import antenv
        antenv.axon_hooks = _m
        _sys.modules["antenv.axon_hooks"] = _m
    except Exception:
        pass

import concourse.bass as bass
import concourse.mybir as mybir
import concourse.tile as tile
from concourse import bacc
from concourse.bass_utils import run_bass_kernel_spmd

F32 = mybir.dt.float32
F32R = mybir.dt.float32r
BF16 = mybir.dt.bfloat16
F8E4 = mybir.dt.float8e4
AFT = mybir.ActivationFunctionType
DR = mybir.MatmulPerfMode.DoubleRow

B = 8
SEQ = 256
NCLS = 313
E = 512
NH = 8
HD = 64
N1 = SEQ + NCLS  # 569
P = 16
IMG = 256

# n/m chunking of the 569 token dim.
# fp32r matmuls require even free counts, so padded widths (..P) are used for
# fp32r operands/psum; real widths for bf16 ops and final stores.
N1P = 570
CHUNKS = [(0, 128), (128, 128), (256, 128), (384, 128), (512, 57)]
CWP = [128, 128, 128, 128, 58]
SPANS = [(0, 512), (512, 58)]

LAST_RESULT = None
_CACHED = {}


def r32(ap):
    if ap.dtype == F32R:
        return ap
    return ap.bitcast(F32R)


def _build_program():
    nc = bacc.Bacc("TRN2", target_bir_lowering=False, debug=False, num_devices=B)

    # ---- DRAM I/O ----
    d_xT = nc.dram_tensor("xT", [E, N1], F32R, kind="ExternalInput").ap()
    d_mask = nc.dram_tensor("mask8", [IMG, IMG * NCLS], F8E4, kind="ExternalInput").ap()
    d_qkvwT = nc.dram_tensor("qkv_wT", [E, 3 * E], F32R, kind="ExternalInput").ap()
    d_owT = nc.dram_tensor("o_wT", [E, E], F32R, kind="ExternalInput").ap()
    d_ob = nc.dram_tensor("o_b", [1, E], F32R, kind="ExternalInput").ap()
    d_tau = nc.dram_tensor("tau", [1, 1], F32, kind="ExternalInput").ap()
    d_bd2 = nc.dram_tensor("bd2", [128, 16 * 2 * 128], F8E4, kind="ExternalInput").ap()
    d_ident = nc.dram_tensor("ident", [128, 128], BF16, kind="ExternalInput").ap()
    d_unitv = nc.dram_tensor("unitv", [128, 512], F32R, kind="ExternalInput").ap()
    d_out = nc.dram_tensor("out", [N1, E], F32, kind="ExternalOutput").ap()

    with tile.TileContext(nc) as tc:
        _emit(nc, tc, d_xT, d_mask, d_qkvwT, d_owT, d_ob, d_tau, d_bd2, d_ident,
              d_unitv, d_out)

    nc.compile()
    return nc


def _emit(nc, tc, d_xT, d_mask, d_qkvwT, d_owT, d_ob, d_tau, d_bd2, d_ident,
          d_unitv, d_out):
    from contextlib import ExitStack

    ctx = ExitStack()
    singles = ctx.enter_context(tc.tile_pool(name="singles", bufs=1))
    expool = ctx.enter_context(tc.tile_pool(name="expT", bufs=40))
    opool = ctx.enter_context(tc.tile_pool(name="outTsb", bufs=8))
    spool = ctx.enter_context(tc.tile_pool(name="smalls", bufs=2))
    ps_work = ctx.enter_context(tc.tile_pool(name="ps_work", bufs=4, space="PSUM"))
    mctx = ExitStack()
    mpool = mctx.enter_context(tc.tile_pool(name="mask_stream", bufs=3))
    ps_mask = mctx.enter_context(tc.tile_pool(name="ps_mask", bufs=2, space="PSUM"))

    # ---- persistent SBUF ----
    inputsT = [singles.tile([128, N1P], F32R, tag=f"inT{i}", name=f"inT{i}") for i in range(4)]
    qkvwT = [singles.tile([128, 3 * E], F32R, tag=f"qkvwT{i}", name=f"qkvwT{i}") for i in range(4)]
    owT2 = [singles.tile([128, E], F32R, tag=f"owT{i}", name=f"owT{i}") for i in range(4)]
    tmpodd = [singles.tile([64, N1P], F32R, tag=f"tmpo{i}", name=f"tmpo{i}") for i in range(4)]
    bd2_sb = singles.tile([128, 16, 2, 128], F8E4, tag="bd2", name="bd2_sb")
    ident_sb = singles.tile([128, 128], BF16, tag="ident", name="ident_sb")
    ones_sb = singles.tile([128, 128], F32R, tag="ones", name="ones_sb")
    unitv_sb = singles.tile([128, 512], F32R, tag="unitv", name="unitv_sb")
    rtau = singles.tile([128, 1], F32, tag="rtau", name="rtau")
    ob_sb = singles.tile([1, E], F32R, tag="ob", name="ob_sb")
    qkT = [singles.tile([128, N1P], F32R, tag=f"qkT{i}", name=f"qkT{i}") for i in range(8)]
    v_sb = [singles.tile([128, NH * (HD + 1)], BF16, tag=f"vsb{i}", name=f"v_sb{i}") for i in range(5)]
    isone = [singles.tile([128, NCLS], BF16, tag=f"iso{i}", name=f"isone{i}") for i in range(2)]
    isoT = [singles.tile([128, SEQ], BF16, tag=f"isoT{i}", name=f"isoT{i}") for i in range(3)]

    # ---- setup DMAs ----
    # bd2 first on the sync queue (first mask matmul gates on it), then the
    # attention operands.
    nc.sync.dma_start(out=bd2_sb, in_=d_bd2)
    nc.sync.dma_start(out=ident_sb, in_=d_ident)
    for i in range(4):
        nc.sync.dma_start(out=inputsT[i][:, :N1], in_=d_xT[i * 128:(i + 1) * 128, :])
        nc.vector.memset(inputsT[i][:, N1:N1P].bitcast(F32), 0.0)
        nc.sync.dma_start(out=qkvwT[i], in_=d_qkvwT[i * 128:(i + 1) * 128, :])
    for g in range(4):
        nc.sync.dma_start(out=owT2[g], in_=d_owT[g * 128:(g + 1) * 128, :])
    nc.sync.dma_start(out=unitv_sb, in_=d_unitv)
    nc.sync.dma_start(out=ob_sb, in_=d_ob)
    nc.vector.memset(ones_sb[:].bitcast(F32), 1.0)
    # broadcast tau to all partitions (step-0 partition AP), then reciprocal
    tau_bc = bass.AP(tensor=d_tau.tensor, offset=d_tau.offset, ap=[[0, 128], [1, 1]])
    tau_sb = singles.tile([128, 1], F32, tag="tau", name="tau_sb")
    nc.scalar.dma_start(out=tau_sb, in_=tau_bc)
    nc.vector.reciprocal(out=rtau, in_=tau_sb)
    neg1 = singles.tile([128, 1], F32, tag="neg1", name="neg1")
    nc.vector.memset(neg1, -1.0)

    # ---- HAM warmup: dense dummy matmuls so the PE p-state ramps toward
    # full clock while the first mask tiles land; garbage to a scratch psum
    # that is never read ----
    scr = singles.tile([128, 640], BF16, tag="scr", name="scr")
    nc.vector.memset(scr, 1.0)
    ps_warm = ps_work.tile([128, 512], F32, tag="pswork", name="ps_warm")
    for _ in range(6):
        nc.tensor.matmul(out=ps_warm, lhsT=scr[:, 0:128], rhs=scr[:, 128:640],
                         start=True, stop=True)

    # ---- attention work units (emitted interleaved with the mask stream) ----
    expT = {}

    def unit_qkvT(fc):
        def go():
            for sp, (s0, sw) in enumerate(SPANS):
                ps = ps_work.tile([128, sw], F32, tag="pswork", name="pswork")
                for ec in range(4):
                    nc.tensor.matmul(
                        out=ps,
                        lhsT=r32(qkvwT[ec][:, fc * 128:(fc + 1) * 128]),
                        rhs=r32(inputsT[ec][:, s0:s0 + sw]),
                        start=(ec == 0), stop=(ec == 3),
                    )
                nc.vector.tensor_copy(out=qkT[fc][:, s0:s0 + sw], in_=ps)
        return go

    def unit_v(mc):
        def go():
            c0, cw = CHUNKS[mc]
            cwp = CWP[mc]
            ps = ps_work.tile([128, E], F32, tag="pswork", name="pswork")
            for ec in range(4):
                nc.tensor.matmul(
                    out=ps[:cwp, :],
                    lhsT=r32(inputsT[ec][:, c0:c0 + cwp]),
                    rhs=r32(qkvwT[ec][:, 2 * E:3 * E]),
                    start=(ec == 0), stop=(ec == 3),
                )
            for h in range(NH):
                nc.vector.tensor_copy(
                    out=v_sb[mc][:cw, h * 65:h * 65 + 64],
                    in_=ps[:cw, h * 64:(h + 1) * 64],
                )
            nc.vector.memset(v_sb[mc][:cw, 64::65], 1.0)
        return go

    def unit_scores(h, mc):
        def go():
            c0, cw = CHUNKS[mc]
            cwp = CWP[mc]
            kt = qkT[4 + h // 2]
            qt = qkT[h // 2]
            hb = 64 * (h % 2)
            et = expool.tile([128, N1P], BF16, tag="expT", name="expT")
            expT[(h, mc)] = et
            for sp, (s0, sw) in enumerate(SPANS):
                ps = ps_work.tile([128, sw], F32, tag="pswork", name="pswork")
                nc.tensor.matmul(
                    out=ps[:cwp, :],
                    lhsT=r32(kt[hb:hb + 64, c0:c0 + cwp]),
                    rhs=r32(qt[hb:hb + 64, s0:s0 + sw]),
                    start=True, stop=True,
                )
                nc.scalar.activation(
                    out=et[:cwp, s0:s0 + sw], in_=ps[:cwp, :],
                    func=AFT.Exp, scale=rtau[:cwp],
                )
        return go

    def unit_mult(h, mc, half=None):
        # half=0: only the patch-query columns 0:128 (rt0-dependent);
        # half=1: columns 128:256. None: full block (mc<2 color queries).
        def go():
            c0, cw = CHUNKS[mc]
            et = expT[(h, mc)]
            if mc < 2:
                nc.vector.tensor_mul(
                    out=et[:cw, SEQ:N1], in0=et[:cw, SEQ:N1], in1=isone[mc])
            else:
                q0 = 0 if half is None else half * 128
                q1 = SEQ if half is None else (half + 1) * 128
                nc.vector.tensor_mul(
                    out=et[:cw, q0:q1], in0=et[:cw, q0:q1],
                    in1=isoT[mc - 2][:cw, q0:q1])
        return go

    # PE-work units; scores for mc=0,1 first so their expT tiles exist by
    # the time isone[0] lands mid-stream
    units = [unit_qkvT(fc) for fc in range(8)]
    units += [unit_v(mc) for mc in range(5)]
    units += [unit_scores(h, mc) for mc in range(2) for h in range(NH)]
    units += [unit_scores(h, mc) for mc in range(2, 5) for h in range(NH)]
    # DVE mask-mult units that only need the rt=0 half of the mask: the full
    # mc=0 block (patch keys 0..127) and the left query half of mc>=2.
    # unit_mult(h, 0) needs expT(h,0) = units[13+h]; emitted mid-stream only
    # after both that unit and emit_isone(0) have been emitted.
    mults_rt0 = [unit_mult(h, 0) for h in range(NH)]
    mults_rt0 += [unit_mult(h, mc, half=0) for mc in range(2, 5) for h in range(NH)]

    # ---- is_one computation (psum -> multiplicative mask) ----
    ps_m = [None, None]

    def emit_isone(i):
        tmp = spool.tile([128, NCLS], F32, tag="isotmp", name="isotmp")
        nc.scalar.activation(out=tmp, in_=ps_m[i], func=AFT.Square, bias=neg1)
        nc.scalar.activation(out=isone[i], in_=tmp, func=AFT.Relu, scale=-1.0, bias=1.0)

    # transpose one is_one half -> isoT columns (c on partitions); emitted
    # mid-stream for i=0, post-stream for i=1
    def emit_isoT(i):
        for j in range(3):
            cw = 57 if j == 2 else 128
            pst = ps_work.tile([128, 128], BF16, tag="pswork", name="pswork_t")
            nc.tensor.transpose(out=pst[:cw, :], in_=isone[i][:, j * 128:j * 128 + cw],
                                identity=ident_sb)
            nc.vector.tensor_copy(out=isoT[j][:cw, i * 128:(i + 1) * 128], in_=pst[:cw, :])

    # ---- the mask stream ----
    # 32 fp8 tiles of [128 rows x 16q x 313c] (0.64MB, 5KB contiguous
    # per-partition lines). The two DoubleRow k-tiles are ADJACENT pixel
    # columns (same patch column, identical bd mapping), so each tile needs
    # only 8 matmuls of K=128x2 into ps_m[rt]; each tile's matmuls share one
    # stationary bd2 variant.
    QO = 16
    n_q = IMG // QO  # 16
    ui = 0
    # DVE mask-mult thunks become emittable once isone[0]/isoT-left exist
    # AND their expT tile has been produced by a scores unit:
    # scores(h, mc) is units[13 + 8*mc + h].
    mult_ready = [(13 + h, unit_mult(h, 0)) for h in range(NH)]
    mult_ready += [(13 + 8 * mc + h, unit_mult(h, mc, half=0))
                   for mc in range(2, 5) for h in range(NH)]
    mi = 0
    iso0_done = False
    for rt in range(2):
        ps_m[rt] = ps_mask.tile([128, NCLS], F32, tag="psmask", name="psmask")
        for Q in range(n_q):
            t = mpool.tile([128, QO, NCLS], F8E4, tag="mstream", name="mstream")
            src = bass.AP(
                tensor=d_mask.tensor,
                offset=d_mask.offset + rt * 128 * IMG * NCLS + Q * QO * NCLS,
                ap=[[IMG * NCLS, 128], [1, QO * NCLS]],
            )
            nc.gpsimd.dma_start(out=t, in_=src)
            for j in range(QO // 2):
                nc.tensor.matmul(
                    out=ps_m[rt],
                    lhsT=bd2_sb[:, Q],
                    rhs=t[:, 2 * j:2 * j + 2, :],
                    start=(Q == 0 and j == 0),
                    stop=(Q == n_q - 1 and j == QO // 2 - 1),
                    perf_mode=DR,
                )
            ti = rt * n_q + Q
            # keep the PE dense: from tile 4 onward interleave attention
            # units (their DMAs on the sync queue land ~12us in)
            if ti >= 4 and ui < len(units):
                budget = 2 if ti >= 8 else 1
                for _ in range(budget):
                    if ui < len(units):
                        units[ui]()
                        ui += 1
            if ti == 18:
                emit_isoT(0)
                iso0_done = True
            if iso0_done:
                while mi < len(mult_ready) and mult_ready[mi][0] < ui:
                    mult_ready[mi][1]()
                    mi += 1
        emit_isone(rt)
    while ui < len(units):
        units[ui]()
        ui += 1
    while mi < len(mult_ready):
        mult_ready[mi][1]()
        mi += 1
    emit_isoT(1)
    mctx.close()
    ps_out = ctx.enter_context(tc.tile_pool(name="ps_out", bufs=1, space="PSUM"))

    # ---- remaining mask-mult + attn@v with gathered denominators ----
    # Per group of 4 heads: rt1-half mask-mults, attn@v into psum (ones
    # column of v gives the softmax denominator in row 64), evacuate the
    # unnormalized outT, and gather the 4 heads' denominator rows at
    # partitions {0,32,64,96} of a shared psum tile via K=1 unit-vector
    # matmuls. One reciprocal per span serves the group; PE broadcasts each
    # head's reciprocal row and DVE normalizes (reading the broadcast psum
    # directly). Heads are stored in PAIRS on 128 partitions (outT2[p]):
    # even head -> partitions 0:64 in place; odd head -> normalized in a
    # staging tile then SB->SB DMA'd to partitions 64:128, so o_proj runs
    # K=128 matmuls.
    outT2 = [opool.tile([128, N1P], F32R, tag="outT", name="outT") for _ in range(4)]
    for g in range(2):
        den_ps = {}
        for sp, (s0, sw) in enumerate(SPANS):
            den_ps[sp] = ps_out.tile([128, sw], F32, tag=f"denps{sp}", name="denps", bufs=1)
        for h4 in range(4):
            h = g * 4 + h4
            unit_mult(h, 1)()
            for mc in range(2, 5):
                unit_mult(h, mc, half=1)()
            rec = spool.tile([65, N1P], F32R, tag="rec", name="rec")
            p, side = h // 2, h % 2
            for sp, (s0, sw) in enumerate(SPANS):
                pso = ps_out.tile([65, sw], F32, tag="psout", name="psout", bufs=2)
                for mc in range(5):
                    c0, cw = CHUNKS[mc]
                    nc.tensor.matmul(
                        out=pso,
                        lhsT=v_sb[mc][:cw, h * 65:(h + 1) * 65],
                        rhs=expT[(h, mc)][:cw, s0:s0 + sw],
                        start=(mc == 0), stop=(mc == 4),
                    )
                with nc.allow_low_precision(reason="f32r copies"):
                    nc.scalar.activation(out=rec[64:65, s0:s0 + sw], in_=pso[64:65, :],
                                         func=AFT.Copy)
                    dst = outT2[p][0:64, s0:s0 + sw] if side == 0 else \
                        tmpodd[p][:, s0:s0 + sw]
                    nc.vector.tensor_copy(out=dst, in_=pso[0:64, :])
                nc.tensor.matmul(
                    out=den_ps[sp],
                    lhsT=r32(unitv_sb[64:65, h4 * 128:(h4 + 1) * 128]),
                    rhs=r32(rec[64:65, s0:s0 + sw]),
                    start=(h4 == 0), stop=(h4 == 3),
                )
        drec = {}
        for sp, (s0, sw) in enumerate(SPANS):
            dr = spool.tile([128, sw], F32R, tag=f"drec{sp}", name=f"drec{sp}")
            with nc.allow_low_precision(reason="f32r reciprocal"):
                nc.vector.reciprocal(out=dr, in_=den_ps[sp])
            drec[sp] = dr
        for h4 in range(4):
            h = g * 4 + h4
            p, side = h // 2, h % 2
            for sp, (s0, sw) in enumerate(SPANS):
                psb = ps_work.tile([64, sw], F32, tag="pswork", name="psb")
                nc.tensor.matmul(
                    out=psb,
                    lhsT=r32(ones_sb[32 * h4:32 * h4 + 1, 0:64]),
                    rhs=drec[sp][32 * h4:32 * h4 + 1, :],
                    start=True, stop=True,
                    tile_position=(32 * h4, 0),
                )
                dst = outT2[p][0:64, s0:s0 + sw] if side == 0 else \
                    tmpodd[p][:, s0:s0 + sw]
                with nc.allow_low_precision(reason="in-place normalize"):
                    nc.vector.tensor_mul(out=dst, in0=dst, in1=psb)
            if side == 1:
                nc.gpsimd.dma_start(out=outT2[p][64:128, :], in_=tmpodd[p])

    # ---- o_proj + bias + store (bias via K=1 ones-row matmul) ----
    for mc in range(5):
        c0, cw = CHUNKS[mc]
        cwp = CWP[mc]
        psf = ps_work.tile([128, E], F32, tag="pswork", name="psf")
        nc.tensor.matmul(
            out=psf[:cwp, :],
            lhsT=r32(ones_sb[0:1, 0:cwp]),
            rhs=ob_sb,
            start=True, stop=False,
        )
        for g4 in range(4):
            nc.tensor.matmul(
                out=psf[:cwp, :],
                lhsT=r32(outT2[g4][:, c0:c0 + cwp]),
                rhs=r32(owT2[g4]),
                start=False, stop=(g4 == 3),
            )
        fin = spool.tile([128, E], F32, tag="fin", name="fin")
        nc.scalar.activation(out=fin[:cw, :], in_=psf[:cw, :], func=AFT.Copy)
        nc.sync.dma_start(out=d_out[c0:c0 + cw, :], in_=fin[:cw, :])

    ctx.close()


def _constants():
    # bd2[w][r, kt, s'] = 1 iff s' = (r//16)*16 + w; both k-tiles (adjacent
    # pixel columns of the same patch) use the identical row->patch mapping.
    bd2 = np.zeros((16, 128, 2, 128), dtype=np.float32)
    r = np.arange(128)
    for w in range(16):
        for kt in range(2):
            bd2[w, r, kt, (r // 16) * 16 + w] = 1.0
    # flatten to the [128, 16*2*128] DRAM layout: partition r, then (w, kt, s')
    bd2 = bd2.transpose(1, 0, 2, 3).reshape(128, 16 * 2 * 128)
    ident = np.eye(128, dtype=ml_dtypes.bfloat16)
    unitv = np.zeros((128, 512), dtype=np.float32)
    for h4 in range(4):
        unitv[:, h4 * 128 + 32 * h4] = 1.0
    return bd2.astype(ml_dtypes.float8_e4m3), ident, unitv


def kernel(x, colors, mask, qkv_w, o_w, o_b, tau):
    global LAST_RESULT
    if "nc" not in _CACHED:
        _CACHED["nc"] = _build_program()
    nc = _CACHED["nc"]

    bd2, ident, unitv = _constants()
    qkv_wT = np.ascontiguousarray(np.asarray(qkv_w, dtype=np.float32).T)
    o_wT = np.ascontiguousarray(np.asarray(o_w, dtype=np.float32).T)
    o_b2 = np.asarray(o_b, dtype=np.float32).reshape(1, E)
    tau2 = np.asarray(tau, dtype=np.float32).reshape(1, 1)

    in_maps = []
    for b in range(B):
        xT = np.ascontiguousarray(
            np.concatenate([np.asarray(x[b]), np.asarray(colors[b])], axis=0).T
        ).astype(np.float32)
        mb = np.asarray(mask[b], dtype=np.float32).reshape(IMG, IMG * NCLS)
        mb8 = mb.astype(ml_dtypes.float8_e4m3)
        in_maps.append({
            "xT": xT, "mask8": mb8, "qkv_wT": qkv_wT, "o_wT": o_wT,
            "o_b": o_b2, "tau": tau2, "bd2": bd2, "ident": ident, "unitv": unitv,
        })

    res = run_bass_kernel_spmd(nc, in_maps, list(range(B)))
    LAST_RESULT = res
    out = np.stack([res.results[i]["out"] for i in range(B)]).astype(np.float32)
    return out
